# revision 1
# baseline (speedup 1.0000x reference)
"""Trainium2 Bass kernel for nn_MixedMlp (soft-mixture MoE MLP) — v3.

Math (per batch row b):
    cn = LayerNorm(c); x = [z, cn]
    coeff = softmax(gateMLP(x))                       # [E]
    l0 = elu(sum_e coeff_e (x @ w0_e + b0_e))
    l1 = elu(sum_e coeff_e ([z, l0] @ w1_e + b1_e))
    out = sum_e coeff_e ([z, l1] @ w2_e + b2_e)

v3 structure (2-block pipeline, 512 rows/block, unnormalized coefficients):
  * Softmax normalization is NOT applied to the coefficients.  Each layer
    is linear in the coefficients, so raw eL = exp(logits) scales the
    inputs, and 1/sum(eL) is applied once per layer to the accumulated
    PSUM (one tensor_tensor per output half, rnb = DMA-broadcast row)
    right before the elu; the final layer's normalization is a single
    per-partition tensor_scalar on the row-major output.  This takes the
    sum/reciprocal chain off the critical path: the eL broadcast DMA
    starts immediately after the gate's last matmul.
  * No Ln/Sqrt tables: rstd = 1/sqrt(var+eps) via cubic poly + 1 Newton
    on DVE.  Only ACT table set 0 (exp/relu/copy), prewarmed at t=0.
  * PE warmup matmuls bridge the input-DMA window so the HAM clock gate
    reaches 2.4GHz before the real layers.
  * Elementwise work split DVE/GPSIMD by measured throughput (gpsimd TT
    is ~660ns flat for <=512-wide tiles); gpsimd ops are emitted in
    execution order to avoid head-of-line blocking in its FIFO queue.
"""

import numpy as np
from contextlib import ExitStack

import concourse.bass as bass
import concourse.bacc as bacc
import concourse.tile as tile
import concourse.mybir as mybir
from concourse import bass_utils
from concourse.bass import AP

F32 = mybir.dt.float32
F16 = mybir.dt.float16
AF = mybir.ActivationFunctionType
OP = mybir.AluOpType

N_CORES = 8
B = 8192
R = B // N_CORES          # rows per core = 1024
LATENT, CIN, HID, ACTD, E, GH = 32, 128, 256, 16, 8, 128
IN0, INTER = LATENT + CIN, HID + LATENT
LN_EPS = 1e-5
BT = 512                  # rows per pipeline block
NBLK = R // BT            # 2
NCH = R // 128            # 8 chunks per core

# rsqrt(1+t) cubic fit on t in [-0.75, 0.75]; one Newton step after.
_tt = np.linspace(-0.75, 0.75, 4001)
_PC3, _PC2, _PC1, _PC0 = [float(c) for c in np.polyfit(_tt, 1.0 / np.sqrt(1.0 + _tt), 3)]

_GCOLS = [("g0z", 128), ("g0c", 128), ("g1w", 128), ("g2w", 8),
          ("b01", 512), ("on8", 1), ("onr", 8), ("i16", 128), ("b2s", 16)]
_WCOLS = [("w0z", 512), ("w0c", 2048), ("w1z", 512), ("w1h", 4096),
          ("w2s", 384), ("s2", 16)]
_GOFF, _WOFF = {}, {}
_o = 0
for _n, _c in _GCOLS:
    _GOFF[_n] = _o
    _o += _c
NGATE = _o
_o = 0
for _n, _c in _WCOLS:
    _WOFF[_n] = _o
    _o += _c
NWALL = _o
NW0 = _WOFF["w1z"]        # w0z + w0c columns (first wall piece)

_CACHE = {}


def _build_program():
    nc = bacc.Bacc("TRN2", target_bir_lowering=False, debug=False,
                   num_devices=N_CORES)

    zr_d = nc.dram_tensor("zrep", [128, R], F16, kind="ExternalInput").ap()
    c_d = nc.dram_tensor("cperm", [128, NCH * CIN], F16, kind="ExternalInput").ap()
    wg_d = nc.dram_tensor("wgate", [128, NGATE], F16, kind="ExternalInput").ap()
    wall_d = nc.dram_tensor("wall", [128, NWALL], F16, kind="ExternalInput").ap()
    ck_d = nc.dram_tensor("consts", [128, 8], F32, kind="ExternalInput").ap()
    out_d = nc.dram_tensor("out", [R, ACTD], F32, kind="ExternalOutput").ap()

    with tile.TileContext(nc) as tc, ExitStack() as ctx:
        wp = ctx.enter_context(tc.tile_pool(name="wp", bufs=1))
        big = ctx.enter_context(tc.tile_pool(name="big", bufs=1))
        cpool = ctx.enter_context(tc.tile_pool(name="cpool", bufs=2))
        cnp = ctx.enter_context(tc.tile_pool(name="cnp", bufs=2))
        cbp = ctx.enter_context(tc.tile_pool(name="cbp", bufs=2))
        zsp = ctx.enter_context(tc.tile_pool(name="zsp", bufs=2))
        s0p = ctx.enter_context(tc.tile_pool(name="s0p", bufs=2))
        ghp = ctx.enter_context(tc.tile_pool(name="ghp", bufs=2))
        er = ctx.enter_context(tc.tile_pool(name="er", bufs=3))
        sc0 = ctx.enter_context(tc.tile_pool(name="sc0", bufs=4))
        sc1 = ctx.enter_context(tc.tile_pool(name="sc1", bufs=4))
        sp8 = ctx.enter_context(tc.tile_pool(name="sp8", bufs=2))
        otp = ctx.enter_context(tc.tile_pool(name="otp", bufs=2))
        pt = ctx.enter_context(tc.tile_pool(name="pt", bufs=2, space="PSUM"))
        pm = ctx.enter_context(tc.tile_pool(name="pm", bufs=4, space="PSUM"))
        psm = ctx.enter_context(tc.tile_pool(name="psm", bufs=2, space="PSUM"))
        dstage = ctx.enter_context(tc.tile_pool(name="dstage", bufs=1, space="DRAM"))

        # ---------------- prologue: prewarm + loads ----------------
        dmy = wp.tile([128, 512], F16)
        nc.gpsimd.memset(dmy[:], 0.25)
        dume = er.tile([128, 64], F16, tag="dume", bufs=1)
        nc.scalar.activation(dume[:], dmy[:, 0:64], AF.Exp)
        nc.scalar.activation(dume[:], dmy[:, 0:64], AF.Relu)

        cts = []
        for k in range(NBLK):
            ct = cpool.tile([128, BT], F16, tag="ct", name=f"ct{k}")
            nc.sync.dma_start(ct[:], c_d[:, BT * k:BT * (k + 1)])
            cts.append(ct)
        zrep = big.tile([128, R], F16)
        nc.sync.dma_start(zrep[:], zr_d[:])
        wgate = wp.tile([128, NGATE], F16)
        nc.sync.dma_start(wgate[:], wg_d[:], max_dma_last_dim=4096)
        ckt = wp.tile([128, 8], F32)
        nc.gpsimd.dma_start(ckt[:], ck_d[:])
        wall = wp.tile([128, NWALL], F16)
        nc.gpsimd.dma_start(wall[:, 0:NW0], wall_d[:, 0:NW0], max_dma_last_dim=4096)
        nc.gpsimd.dma_start(wall[:, NW0:NWALL], wall_d[:, NW0:NWALL],
                            max_dma_last_dim=4096)

        # PE warmup: bridge the DMA window so HAM un-throttles early
        wps = pm.tile([128, 512], F32, tag="mm", name="warm")
        for i in range(8):
            nc.tensor.matmul(wps[:], dmy[:, 0:128], dmy[:],
                             start=True, stop=True)

        def wsl(name, p0, pn, c0, cn_):
            if name in _GOFF:
                o = _GOFF[name]
                return wgate[p0:p0 + pn, o + c0:o + c0 + cn_]
            o = _WOFF[name]
            return wall[p0:p0 + pn, o + c0:o + c0 + cn_]

        lng, lnb = ckt[:, 0:1], ckt[:, 1:2]
        g0b, g1b = ckt[:, 2:3], ckt[:, 3:4]
        g2b = ckt[0:8, 4:5]

        # ---------------- persistent tiles ----------------
        eL = big.tile([8, R], F16)          # exp(gate logits)
        coeffN = big.tile([8, R], F16)      # softmax coefficients
        mv = big.tile([128, 16], F32)
        rstd = big.tile([128, 8], F32)
        cstage = dstage.tile([8, R], F16)
        ctens = cstage.tensor

        FR = [dict() for _ in range(NBLK)]

        # ---------------- front-end for one block ----------------
        def front(k):
            bs = slice(BT * k, BT * (k + 1))
            ct = cts[k]
            for jj in range(4):
                j = 4 * k + jj
                st = sp8.tile([128, 6], F32, tag="st", name=f"st{j}")
                nc.vector.bn_stats(st[:], ct[:, 128 * jj:128 * (jj + 1)])
                nc.vector.bn_aggr(mv[:, 2 * j:2 * j + 2], st[:])
            # rstd = 1/sqrt(var+eps): cubic poly + 1 Newton, DVE only
            var4 = AP(mv[:].tensor, mv[:].offset + 8 * k + 1,
                      [list(mv[:].ap[0]), [2, 4]])
            t_ = sp8.tile([128, 4], F32, tag="pt", name=f"pt{k}")
            nc.vector.tensor_scalar_sub(t_[:], var4, 1.0 - LN_EPS)
            ea = sp8.tile([128, 4], F32, tag="pa", name=f"pa{k}")
            nc.vector.tensor_scalar(ea[:], t_[:], _PC1, _PC0, OP.mult, OP.add)
            eb = sp8.tile([128, 4], F32, tag="pb", name=f"pb{k}")
            nc.vector.tensor_scalar(eb[:], t_[:], _PC3, _PC2, OP.mult, OP.add)
            t2 = sp8.tile([128, 4], F32, tag="pc", name=f"pc{k}")
            nc.vector.scalar_tensor_tensor(t2[:], t_[:], 0.0, t_[:], OP.add, OP.mult)
            ebt = sp8.tile([128, 4], F32, tag="pd", name=f"pd{k}")
            nc.vector.scalar_tensor_tensor(ebt[:], eb[:], 0.0, t2[:], OP.add, OP.mult)
            y0 = sp8.tile([128, 4], F32, tag="pe", name=f"pe{k}")
            nc.vector.tensor_add(y0[:], ea[:], ebt[:])
            vpe = sp8.tile([128, 4], F32, tag="pf", name=f"pf{k}")
            nc.vector.tensor_scalar_add(vpe[:], t_[:], 1.0)
            ysq = sp8.tile([128, 4], F32, tag="pg", name=f"pg{k}")
            nc.vector.scalar_tensor_tensor(ysq[:], y0[:], 0.0, y0[:], OP.add, OP.mult)
            vy2 = sp8.tile([128, 4], F32, tag="ph", name=f"ph{k}")
            nc.vector.scalar_tensor_tensor(vy2[:], ysq[:], 0.0, vpe[:], OP.add, OP.mult)
            hc = sp8.tile([128, 4], F32, tag="pi", name=f"pi{k}")
            nc.vector.tensor_scalar(hc[:], vy2[:], -0.5, 1.5, OP.mult, OP.add)
            nc.vector.scalar_tensor_tensor(rstd[:, 4 * k:4 * k + 4], y0[:], 0.0,
                                           hc[:], OP.add, OP.mult)

            cn = cnp.tile([128, BT], F16, tag="cn", name=f"cn{k}")
            for jj in range(4):
                j = 4 * k + jj
                y = er.tile([128, 128], F16, tag="y", name=f"y{j}")
                nc.vector.tensor_scalar(y[:], ct[:, 128 * jj:128 * (jj + 1)],
                                        mv[:, 2 * j:2 * j + 1],
                                        rstd[:, j:j + 1], OP.subtract, OP.mult)
                yT = pt.tile([128, 128], F16, tag="tp", name=f"yT{j}")
                nc.tensor.transpose(yT[:], y[:], wsl("i16", 0, 128, 0, 128))
                nc.scalar.activation(cn[:, 128 * jj:128 * (jj + 1)], yT[:],
                                     AF.Identity, bias=lnb, scale=lng)

            # gate MLP: two 256-wide sub-chains (pipeline PE/ACT/DVE, halve
            # the exposed latency of the block-0 front-end)
            for sh in range(2):
                ss = slice(BT * k + 256 * sh, BT * k + 256 * (sh + 1))
                cns = cn[:, 256 * sh:256 * (sh + 1)]
                gp = pm.tile([128, 512], F32, tag="mm", name=f"gp{k}_{sh}")
                pre0 = gp[:, 0:256]
                nc.tensor.matmul(pre0, wsl("g0z", 0, 32, 0, 128), zrep[0:32, ss],
                                 start=True, stop=False)
                nc.tensor.matmul(pre0, wsl("g0c", 0, 128, 0, 128), cns,
                                 start=False, stop=True)
                e0 = er.tile([128, 256], F16, tag="ge", name=f"ge0{k}{sh}")
                nc.scalar.activation(e0[:], pre0, AF.Exp, bias=g0b)
                r0 = er.tile([128, 256], F16, tag="gr", name=f"gr0{k}{sh}")
                nc.scalar.activation(r0[:], pre0, AF.Relu, bias=g0b)
                h0 = ghp.tile([128, 256], F16, tag="h0", name=f"h0{k}{sh}")
                nc.vector.scalar_tensor_tensor(h0[:], e0[:], 1.0, r0[:],
                                               OP.min, OP.add)
                pre1 = gp[:, 256:512]
                nc.tensor.matmul(pre1, wsl("g1w", 0, 128, 0, 128), h0[:],
                                 start=True, stop=True)
                e1 = er.tile([128, 256], F16, tag="ge", name=f"ge1{k}{sh}")
                nc.scalar.activation(e1[:], pre1, AF.Exp, bias=g1b)
                r1 = er.tile([128, 256], F16, tag="gr", name=f"gr1{k}{sh}")
                nc.scalar.activation(r1[:], pre1, AF.Relu, bias=g1b)
                h1 = ghp.tile([128, 256], F16, tag="h1", name=f"h1{k}{sh}")
                nc.vector.scalar_tensor_tensor(h1[:], e1[:], 1.0, r1[:],
                                               OP.min, OP.add)
                smt = psm.tile([8, 512], F32, tag="sm", name=f"smt{k}{sh}")
                pre2 = smt[0:8, 0:256]
                nc.tensor.matmul(pre2, wsl("g2w", 0, 128, 0, 8), h1[:],
                                 start=True, stop=True)
                nc.scalar.activation(eL[:, ss], pre2, AF.Exp, bias=g2b)
                sume = smt[0:1, 256:512]
                nc.tensor.matmul(sume, wsl("on8", 0, 8, 0, 1), eL[:, ss],
                                 start=True, stop=True)
                rsum = sp8.tile([1, 256], F32, tag="rsm", name=f"rsum{k}{sh}")
                nc.vector.reciprocal_approx_fast(rsum[:], sume)
                rsr = sp8.tile([1, 256], F16, tag="rsr", name=f"rsr{k}{sh}")
                nc.vector.tensor_copy(rsr[:], rsum[:])
                rbc = smt[0:8, 0:256]
                nc.tensor.matmul(rbc, wsl("onr", 0, 1, 0, 8), rsr[:],
                                 start=True, stop=True)
                nc.vector.tensor_mul(coeffN[:, ss], eL[:, ss], rbc)
                nc.sync.dma_start(cstage[0:8, ss], coeffN[:, ss])

            # per-block broadcasts of the staged coefficients
            eball = cbp.tile([128, E * BT], F16, tag="eb", name=f"eball{k}")
            for eh in range(2):
                q = nc.sync if eh == 0 else nc.scalar
                q.dma_start(
                    AP(eball.tensor, E * BT // 2 * eh,
                       [[E * BT, 128], [BT, 4], [1, BT]]),
                    AP(ctens, 4 * eh * R + BT * k, [[0, 128], [R, 4], [1, BT]]))
            cbz = cbp.tile([128, 2 * BT], F16, tag="cbz", name=f"cbz{k}")
            for q_ in range(2):
                nc.scalar.dma_start(
                    cbz[:, BT * q_:BT * (q_ + 1)],
                    AP(ctens, 4 * q_ * R + BT * k, [[R, 4], [0, 32], [1, BT]]))
            cbe = cbp.tile([128, BT], F16, tag="cbe", name=f"cbe{k}")
            nc.scalar.dma_start(
                cbe[:], AP(ctens, BT * k, [[R, 8], [0, 16], [1, BT]]))
            FR[k].update(cn=cn, eball=eball, cbz=cbz, cbe=cbe)

        def norm_elu(k, ps, dst, tagsuf):
            ee = er.tile([128, 512], F16, tag="ee", name=f"ee{tagsuf}")
            nc.scalar.activation(ee[:], ps[:], AF.Exp)
            rr = er.tile([128, 512], F16, tag="rr", name=f"rr{tagsuf}")
            nc.scalar.activation(rr[:], ps[:], AF.Relu)
            nc.vector.scalar_tensor_tensor(dst, ee[:], 1.0, rr[:], OP.min, OP.add)

        # ---------------- expert layer 0 ----------------
        def l0(k):
            fr = FR[k]
            bs = slice(BT * k, BT * (k + 1))
            zs = zsp.tile([128, 2 * BT], F16, tag="zs", name=f"zs{k}")
            for q_ in range(2):
                nc.gpsimd.tensor_mul(zs[:, BT * q_:BT * (q_ + 1)], zrep[:, bs],
                                     fr["cbz"][:, BT * q_:BT * (q_ + 1)])
            fr["zs"] = zs
            ps = [pm.tile([128, 512], F32, tag="mm", name=f"l0p{k}_{mt}")[:]
                  for mt in range(2)]
            for mt in range(2):
                nc.tensor.matmul(ps[mt], wsl("b01", 0, 8, 128 * mt, 128),
                                 coeffN[:, bs], start=True, stop=False)
            for e in range(E):
                t0 = sc0.tile([128, 512], F16, tag="t0", name=f"t0_{k}_{e}")
                nc.vector.tensor_mul(t0[:], fr["cn"][:],
                                     fr["eball"][:, BT * e:BT * (e + 1)])
                for mt in range(2):
                    nc.tensor.matmul(ps[mt],
                                     wsl("w0c", 0, 128, 256 * e + 128 * mt, 128),
                                     t0[:], start=False, stop=False)
            for q_ in range(2):
                for mt in range(2):
                    nc.tensor.matmul(ps[mt],
                                     wsl("w0z", 0, 128, 256 * q_ + 128 * mt, 128),
                                     zs[:, BT * q_:BT * (q_ + 1)],
                                     start=False, stop=(q_ == 1))
            fr["ps0"] = ps

        # ---------------- expert layer 1 ----------------
        def l1(k):
            fr = FR[k]
            bs = slice(BT * k, BT * (k + 1))
            ps = [pm.tile([128, 512], F32, tag="mm", name=f"l1p{k}_{mt}")[:]
                  for mt in range(2)]
            for mt in range(2):
                nc.tensor.matmul(ps[mt], wsl("b01", 0, 8, 256 + 128 * mt, 128),
                                 coeffN[:, bs], start=True, stop=False)
            for e in range(E):
                t1 = sc1.tile([128, 2 * BT], F16, tag="t1", name=f"t1_{k}_{e}")
                nc.vector.tensor_mul(t1[:], fr["s0"][:],
                                     AP(fr["eball"].tensor, BT * e,
                                        [[E * BT, 128], [0, 2], [1, BT]]))
                for h in range(2):
                    for mt in range(2):
                        nc.tensor.matmul(
                            ps[mt],
                            wsl("w1h", 0, 128, 256 * (2 * e + h) + 128 * mt, 128),
                            t1[:, BT * h:BT * (h + 1)],
                            start=False, stop=False)
            for q_ in range(2):
                for mt in range(2):
                    nc.tensor.matmul(ps[mt],
                                     wsl("w1z", 0, 128, 256 * q_ + 128 * mt, 128),
                                     fr["zs"][:, BT * q_:BT * (q_ + 1)],
                                     start=False, stop=(q_ == 1))
            fr["ps1"] = ps

        def e0(k):
            fr = FR[k]
            s0 = s0p.tile([128, 2 * BT], F16, tag="s0", name=f"s0_{k}")
            for mt in range(2):
                norm_elu(k, fr["ps0"][mt], s0[:, BT * mt:BT * (mt + 1)],
                         f"0_{k}_{mt}")
            fr["s0"] = s0

        def e1(k):
            fr = FR[k]
            s1 = s0p.tile([128, 2 * BT], F16, tag="s1", name=f"s1_{k}")
            for mt in range(2):
                norm_elu(k, fr["ps1"][mt], s1[:, BT * mt:BT * (mt + 1)],
                         f"1_{k}_{mt}")
            fr["s1"] = s1

        # ---------------- expert layer 2 + output ----------------
        def l2(k):
            fr = FR[k]
            bs = slice(BT * k, BT * (k + 1))
            per2 = pm.tile([128, 512], F32, tag="mm", name=f"l2p{k}")[:]
            nc.tensor.matmul(per2, wsl("w2s", 0, 32, 0, 128), zrep[0:32, bs],
                             start=True, stop=False)
            nc.tensor.matmul(per2, wsl("w2s", 0, 128, 128, 128),
                             fr["s1"][:, 0:BT], start=False, stop=False)
            nc.tensor.matmul(per2, wsl("w2s", 0, 128, 256, 128),
                             fr["s1"][:, BT:2 * BT], start=False, stop=True)
            mixed = er.tile([128, 512], F16, tag="mx", name=f"mx{k}")
            nc.vector.tensor_mul(mixed[:], per2, fr["cbe"][:])
            otb = otp.tile([128, 64], F32, tag="ot", name=f"ot{k}")
            for jj in range(4):
                po = pt.tile([128, 16], F32, tag="tp", name=f"po{k}_{jj}")
                nc.tensor.matmul(po[:],
                                 mixed[:, 128 * jj:128 * (jj + 1)],
                                 wsl("s2", 0, 128, 0, 16), start=True, stop=False)
                nc.tensor.matmul(po[:],
                                 coeffN[:, BT * k + 128 * jj:BT * k + 128 * (jj + 1)],
                                 wsl("b2s", 0, 8, 0, 16), start=False, stop=True)
                nc.vector.tensor_copy(otb[:, 16 * jj:16 * (jj + 1)], po[:])
            nc.sync.dma_start(AP(out_d.tensor, 64 * k, [[128, 128], [1, 64]]),
                              otb[:])

        # ---------------- pipelined emission ----------------
        front(0)
        front(1)
        l0(0)
        e0(0)
        l1(0)
        l0(1)
        e0(1)
        e1(0)
        l1(1)
        l2(0)
        e1(1)
        l2(1)

    nc.compile()
    return nc


def _host_prep(inputs):
    f = lambda a: np.ascontiguousarray(np.asarray(a, dtype=np.float32))
    w0, b0 = f(inputs["w0"]), f(inputs["b0"])
    w1, b1 = f(inputs["w1"]), f(inputs["b1"])
    w2, b2 = f(inputs["w2"]), f(inputs["b2"])
    g0w, g0b = f(inputs["g0w"]), f(inputs["g0b"])
    g1w, g1b = f(inputs["g1w"]), f(inputs["g1b"])
    g2w, g2b = f(inputs["g2w"]), f(inputs["g2b"])
    ln_g, ln_b = f(inputs["ln_g"]), f(inputs["ln_b"])

    def ksb(wstk, nkt, m):   # [nkt*128, m] -> [128, nkt*m]
        return np.ascontiguousarray(
            wstk.reshape(nkt, 128, m).transpose(1, 0, 2).reshape(128, nkt * m))

    wall = np.zeros((128, NWALL), np.float32)
    wgate = np.zeros((128, NGATE), np.float32)

    def put(name, arr):
        if name in _GOFF:
            o = _GOFF[name]
            wgate[:arr.shape[0], o:o + arr.shape[1]] = arr
        else:
            o = _WOFF[name]
            wall[:arr.shape[0], o:o + arr.shape[1]] = arr

    put("w0z", ksb(w0[:, :LATENT, :].reshape(E * LATENT, HID), 2, HID))
    put("w0c", ksb(w0[:, LATENT:, :].reshape(E * CIN, HID), 8, HID))
    put("w1z", ksb(w1[:, :LATENT, :].reshape(E * LATENT, HID), 2, HID))
    put("w1h", ksb(w1[:, LATENT:, :].reshape(E * HID, HID), 16, HID))
    w2stk = w2.transpose(1, 0, 2).reshape(INTER, E * ACTD)   # [288, 128]
    w2s = np.zeros((128, 384), np.float32)
    w2s[:32, 0:128] = w2stk[0:32]
    w2s[:, 128:256] = w2stk[32:160]
    w2s[:, 256:384] = w2stk[160:288]
    put("w2s", w2s)
    put("s2", np.tile(np.eye(ACTD, dtype=np.float32), (E, 1)))
    put("g0z", g0w[:LATENT])
    put("g0c", g0w[LATENT:])
    put("g1w", g1w)
    put("g2w", g2w)
    b1f = b1 - w1[:, LATENT:, :].sum(axis=1)
    put("b01", np.concatenate([b0, b1f], axis=1))
    put("on8", np.ones((8, 1), np.float32))
    put("onr", np.ones((1, 8), np.float32))
    put("i16", np.eye(128, dtype=np.float32))
    b2f = b2 - w2[:, LATENT:, :].sum(axis=1)                 # [8,16]
    put("b2s", b2f)

    consts = np.zeros((128, 8), np.float32)
    consts[:, 0] = ln_g
    consts[:, 1] = ln_b
    consts[:, 2] = g0b
    consts[:, 3] = g1b - g1w.sum(0)
    consts[:8, 4] = (g2b - g2w.sum(0))
    return {"wall": wall.astype(np.float16), "wgate": wgate.astype(np.float16),
            "consts": consts}


def make_in_maps(inputs):
    wmap = _host_prep(inputs)
    z = np.ascontiguousarray(np.asarray(inputs["z"], dtype=np.float32))
    c = np.ascontiguousarray(np.asarray(inputs["c"], dtype=np.float32))
    # on-chip batch order: i = 128*r + p  <->  original row b = 8p + r
    ii = np.arange(R)
    perm = 8 * (ii % 128) + ii // 128
    in_maps = []
    for i in range(N_CORES):
        m = dict(wmap)
        zsh = z[i * R:(i + 1) * R]
        m["zrep"] = np.ascontiguousarray(
            np.tile(zsh.T[:, perm], (4, 1))).astype(np.float16)
        csh = c[i * R:(i + 1) * R]
        # partition p <- rows 8p..8p+8 (contiguous lines)
        m["cperm"] = np.ascontiguousarray(
            csh.reshape(128, NCH * CIN)).astype(np.float16)
        in_maps.append(m)
    return in_maps


def kernel(**inputs):
    if "nc" not in _CACHE:
        _CACHE["nc"] = _build_program()
    nc = _CACHE["nc"]
    in_maps = make_in_maps(inputs)
    res = bass_utils.run_bass_kernel_spmd(nc, in_maps, core_ids=list(range(N_CORES)))
    return np.concatenate([res.results[i]["out"] for i in range(N_CORES)], axis=0)

